# revision 5
# baseline (speedup 1.0000x reference)
"""Trainium2 Bass kernel for the autoregressive LSTM problem.

Model (per reference):
  128 warmup LSTM steps over inputs [B=2048, T=128, F=64], U=512 hidden,
  then 32 autoregressive decode steps through a dense head [U, F].

Strategy (fast path, requires b == 0 and dense_b == 0, which the problem's
setup_inputs always produces — a general-bias fallback build is kept below):
  - Data parallel over 8 NeuronCores: 256 batch per core, weights replicated.
  - Transposed layout [feature, batch] on-chip: z^T [2048, 256] accumulates in
    PSUM as 16 m-slices (128 gate-rows each) across 8 banks of [128, 512].
  - bf16 weights / x / h (full PE rate + fast weight load), fp32 PSUM + c.
  - Warmup x-projection: K=64, so two m-slices are computed CONCURRENTLY via
    row tiling (tile_position rows 0-63 / 64-127) — 8 pair-slots instead of
    16 matmul slots per step. The x pairs open each PSUM accumulation group.
  - Decode folds pred away: z_t = h @ (W_h + dense_W @ W_x); no bias matmul
    (b_dec = 0 on the fast path). The dense head for output j runs inside
    decode step t_warm+j's stream (same h read buffer), borrowing the f-gate
    half-0 PSUM bank right after its gate ACT frees it; h never hits DRAM.
  - Stream order per step: x-pairs, k0 sweep, k1 sweep, then per-region
    (k2,k3) tails with gates ordered f,i,g,o per unit-half so the gate ACT +
    c/h update chain overlaps the tail of the matmul stream (f first because
    c = f*c leads the dependency chain).
"""

import numpy as np

B = 2048
T = 128
F = 64
U = 512
OUT_STEPS = 32
N_CORES = 8
BL = B // N_CORES  # per-core batch (= matmul N)

_CACHE = {}

# m-slice index: m = 4*gate + k  (gate 0=i,1=f,2=g,3=o; k = 128-unit chunk)
# PSUM tensor zt[half][gate] ([128, 512]) holds m = 4*gate + 2*half + q.
GATE_ORDER = (1, 0, 2, 3)  # f, i, g, o


def build_nc(t_warm=T, t_dec=OUT_STEPS - 1, bl=BL, reps=None, x_pack=True,
             bf16_gates=False):
    """Fast-path build (assumes b == 0, dense_b == 0). Returns nc.

    reps: wrap the whole compute in a hardware For_i loop for timing.
    """
    import contextlib

    import concourse.bass as bass  # noqa: F401
    import concourse.mybir as mybir
    import concourse.tile as tile
    from concourse import bacc

    f32 = mybir.dt.float32
    bf16 = mybir.dt.bfloat16
    gdt = bf16 if bf16_gates else f32
    AF = mybir.ActivationFunctionType
    n_out = t_dec + 1

    nc = bacc.Bacc("TRN2", target_bir_lowering=False, debug=False,
                   num_devices=N_CORES)

    # DRAM parameters (per core)
    xT_d = nc.dram_tensor("xT", [t_warm, F, bl], bf16,
                          kind="ExternalInput").ap()
    wx_d = nc.dram_tensor("wx_dup", [128, 4 * U], bf16,
                          kind="ExternalInput").ap()
    wh_d = nc.dram_tensor("wh", [U, 4 * U], bf16, kind="ExternalInput").ap()
    whd_d = nc.dram_tensor("wh_dec", [U, 4 * U], bf16,
                           kind="ExternalInput").ap()
    dw_d = nc.dram_tensor("dense_W", [U, F], bf16, kind="ExternalInput").ap()
    out_d = nc.dram_tensor("outT", [n_out, F, bl], f32,
                           kind="ExternalOutput").ap()

    with tile.TileContext(nc) as tc:
        with (
            tc.tile_pool(name="wpool", bufs=1) as wpool,
            tc.tile_pool(name="state", bufs=1) as state,
        ):
            # ---- weights straight to bf16 SBUF ----
            wh_sb = wpool.tile([128, 4, 4 * U], bf16)
            nc.sync.dma_start(out=wh_sb,
                              in_=wh_d.rearrange("(k p) n -> p k n", p=128))
            whd_sb = wpool.tile([128, 4, 4 * U], bf16)
            nc.sync.dma_start(out=whd_sb,
                              in_=whd_d.rearrange("(k p) n -> p k n", p=128))
            wx_sb = wpool.tile([128, 4 * U], bf16)
            nc.sync.dma_start(out=wx_sb, in_=wx_d[:, :])
            dw_sb = wpool.tile([128, 4, F], bf16)
            nc.sync.dma_start(out=dw_sb,
                              in_=dw_d.rearrange("(k p) n -> p k n", p=128))

            # ---- persistent state ----
            # h double-buffered by step parity: step t reads h_bufs[t % 2]
            # (h from step t-1), writes h_bufs[(t+1) % 2]; chunk k at cols
            # [k*bl:(k+1)*bl].
            c_sb = state.tile([128, 4 * bl], f32)
            h_a = state.tile([128, 4 * bl], bf16)
            h_b = state.tile([128, 4 * bl], bf16)
            h_bufs = [h_a, h_b]

            with (
                tc.tile_pool(name="zps", bufs=1, space="PSUM") as zps,
                tc.tile_pool(name="gates", bufs=3) as gates,
                tc.tile_pool(name="tmp", bufs=6) as tmp,
                tc.tile_pool(name="xf", bufs=8) as xf_pool,
                tc.tile_pool(name="po", bufs=4) as po,
                tc.For_i(0, reps) if reps else contextlib.nullcontext(),
            ):
                nc.vector.memset(c_sb, 0.0)
                nc.vector.memset(h_a, 0.0)
                x_tiles = {}

                def fetch_x(t):
                    if t >= t_warm:
                        return
                    x_sb = xf_pool.tile([128, bl], bf16, tag="xf",
                                        name=f"xf{t}")
                    nc.sync.dma_start(out=x_sb[0:F, :], in_=xT_d[t])
                    if x_pack:
                        nc.sync.dma_start(out=x_sb[F:2 * F, :], in_=xT_d[t])
                    x_tiles[t] = x_sb

                def step(t, warm):
                    """One LSTM step; t is the global step index. Decode
                    steps also emit the dense head for output t - t_warm."""
                    wh = wh_sb if warm else whd_sb
                    x_sb = x_tiles.pop(t) if warm else None
                    h_rd = h_bufs[t % 2]
                    h_wr = h_bufs[(t + 1) % 2]
                    pred_j = (t - t_warm) if not warm else None
                    z = [[zps.tile([128, 2 * bl], f32, tag=f"z{half}{g}",
                                   name=f"z{half}{g}_{t}")
                          for g in range(4)] for half in range(2)]

                    def zm(m):
                        g, k = m // 4, m % 4
                        half, q = k // 2, k % 2
                        return z[half][g][:, q * bl:(q + 1) * bl]

                    # start=True clears has_written for the WHOLE bank, so
                    # exactly one opener per bank: its q=0 x-MM in warm
                    # steps, its q=0 k0-MM in decode steps.
                    def hmm(m, k, stop=False):
                        ms = slice(m * 128, (m + 1) * 128)
                        nc.tensor.matmul(zm(m), wh[:, k, ms],
                                         h_rd[:, k * bl:(k + 1) * bl],
                                         start=(k == 0 and not warm
                                                and m % 2 == 0),
                                         stop=stop)

                    def xmm_pair(mA, mB):
                        # two K=64 matmuls row-tiled at partitions 0/64;
                        # they target different PSUM banks so they stream
                        # concurrently through the PE array.
                        nc.tensor.matmul(
                            zm(mA), wx_sb[0:F, mA * 128:(mA + 1) * 128],
                            x_sb[0:F, :], start=(mA % 2 == 0), stop=False)
                        nc.tensor.matmul(
                            zm(mB), wx_sb[F:2 * F, mB * 128:(mB + 1) * 128],
                            x_sb[F:2 * F, :], start=(mB % 2 == 0), stop=False)

                    def xmm_single(m):
                        ms = slice(m * 128, (m + 1) * 128)
                        nc.tensor.matmul(zm(m), wx_sb[0:F, ms], x_sb[0:F, :],
                                         start=(m % 2 == 0), stop=False)

                    # tail m-order: per half, gates f,i,g,o, each (q=0, q=1)
                    mord = [[4 * g + 2 * half + q for g in GATE_ORDER
                             for q in range(2)] for half in range(2)]

                    # ---- PE stream ----
                    # x first (h-free prefix covers the h_A(t-1) chain tail)
                    if warm:
                        if x_pack:
                            for j in range(8):
                                xmm_pair(mord[0][j], mord[1][j])
                        else:
                            for m in mord[0] + mord[1]:
                                xmm_single(m)
                    for k in (0, 1):
                        for m in mord[0] + mord[1]:
                            hmm(m, k)
                    for m in mord[0]:
                        hmm(m, 2)
                        hmm(m, 3, stop=(m % 2 == 1))
                    pps = None
                    if pred_j is not None:
                        # dense head for output pred_j: h from h_rd (written
                        # by step t-1; for j=0 the warm final h). Borrows the
                        # f/half-0 bank (z[0][1]) after its gate ACT reads it.
                        pps = zps.tile([F, bl], f32, tag="z01",
                                       name=f"pps{pred_j}")
                        for k in range(4):
                            nc.tensor.matmul(pps, dw_sb[:, k, :],
                                             h_rd[:, k * bl:(k + 1) * bl],
                                             start=(k == 0), stop=(k == 3))
                    for m in mord[1]:
                        hmm(m, 2)
                        hmm(m, 3, stop=(m % 2 == 1))

                    # ---- gate ACTs + state update, per half ----
                    gt = {}
                    for name_, g in (("f", 1), ("i", 0), ("g", 2), ("o", 3)):
                        gt[name_] = gates.tile([128, 4 * bl], gdt,
                                               tag=f"{name_}g",
                                               name=f"{name_}g{t}")
                    for half in range(2):
                        s = slice(half * 2 * bl, (half + 1) * 2 * bl)
                        nc.scalar.activation(gt["f"][:, s], z[half][1],
                                             AF.Sigmoid)
                        nc.scalar.activation(gt["i"][:, s], z[half][0],
                                             AF.Sigmoid)
                        nc.scalar.activation(gt["g"][:, s], z[half][2],
                                             AF.Tanh)
                        nc.scalar.activation(gt["o"][:, s], z[half][3],
                                             AF.Sigmoid)
                        nc.vector.tensor_mul(c_sb[:, s], gt["f"][:, s],
                                             c_sb[:, s])
                        t1 = tmp.tile([128, 2 * bl], f32, tag="t1",
                                      name=f"t1_{t}_{half}")
                        nc.vector.tensor_mul(t1, gt["i"][:, s], gt["g"][:, s])
                        nc.vector.tensor_add(c_sb[:, s], c_sb[:, s], t1)
                        if half == 0 and pps is not None:
                            # pred copy sits here in the DVE FIFO: pps is
                            # ready well before, and h_A below waits on tanh
                            # (ACT) anyway — no chain delay, bank freed early.
                            p_sb = po.tile([F, bl], f32, tag="po",
                                           name=f"po{pred_j}")
                            nc.vector.tensor_copy(p_sb, pps)
                            nc.gpsimd.dma_start(out=out_d[pred_j], in_=p_sb)
                        tch = tmp.tile([128, 2 * bl], gdt, tag="tc",
                                       name=f"tc_{t}_{half}")
                        nc.scalar.activation(tch, c_sb[:, s], AF.Tanh)
                        nc.vector.tensor_mul(h_wr[:, s], gt["o"][:, s], tch)
                    return h_wr

                # warmup
                fetch_x(0)
                fetch_x(1)
                for t in range(t_warm):
                    step(t, warm=True)
                    fetch_x(t + 2)
                # decode: step t_warm+j also emits the dense head for
                # output j (reading the same h buffer the step reads)
                for t in range(t_warm, t_warm + t_dec):
                    step(t, warm=False)
                # final output: h written by the last decode step
                h_last = h_bufs[(t_warm + t_dec) % 2]
                pps = zps.tile([F, bl], f32, tag="z01", name=f"pps{t_dec}")
                for k in range(4):
                    nc.tensor.matmul(pps, dw_sb[:, k, :],
                                     h_last[:, k * bl:(k + 1) * bl],
                                     start=(k == 0), stop=(k == 3))
                p_sb = po.tile([F, bl], f32, tag="po", name=f"po{t_dec}")
                nc.vector.tensor_copy(p_sb, pps)
                nc.gpsimd.dma_start(out=out_d[t_dec], in_=p_sb)

    nc.compile()
    return nc


def prep_inputs(inputs, W_x, W_h, b, dense_W, dense_b, t_warm=T, bl=BL):
    """Host-side prep for the fast path: returns per-core input maps."""
    import ml_dtypes

    bf16 = ml_dtypes.bfloat16
    n_cores = inputs.shape[0] // bl
    W_x = np.asarray(W_x, np.float32)
    W_h = np.asarray(W_h, np.float32)
    dense_W = np.asarray(dense_W, np.float32)

    wh_dec = (W_h.astype(np.float64)
              + dense_W.astype(np.float64) @ W_x.astype(np.float64)
              ).astype(np.float32)
    wx_dup = np.concatenate([W_x, W_x], axis=0)  # [128, 2048]

    shared = {
        "wx_dup": wx_dup.astype(bf16),
        "wh": W_h.astype(bf16),
        "wh_dec": wh_dec.astype(bf16),
        "dense_W": dense_W.astype(bf16),
    }
    in_maps = []
    x = np.asarray(inputs, np.float32)
    for c in range(n_cores):
        shard = x[c * bl:(c + 1) * bl, :t_warm]               # [bl, t, F]
        xT = np.ascontiguousarray(shard.transpose(1, 2, 0))   # [t, F, bl]
        in_maps.append({"xT": xT.astype(bf16), **shared})
    return in_maps


def gather_output(results, bl=BL):
    """results: list of per-core dicts with outT [n_out, F, bl]."""
    outs = []
    for r in results:
        outs.append(np.ascontiguousarray(r["outT"].transpose(2, 0, 1)))
    return np.concatenate(outs, axis=0)  # [B, out_steps, F]


def kernel(inputs, W_x, W_h, b, dense_W, dense_b):
    from concourse.bass_utils import run_bass_kernel_spmd

    if np.any(np.asarray(b)) or np.any(np.asarray(dense_b)):
        return _kernel_bias(inputs, W_x, W_h, b, dense_W, dense_b)
    if "nc" not in _CACHE:
        _CACHE["nc"] = build_nc()
    nc = _CACHE["nc"]
    in_maps = prep_inputs(inputs, W_x, W_h, b, dense_W, dense_b)
    res = run_bass_kernel_spmd(nc, in_maps, core_ids=list(range(N_CORES)),
                               trace=False)
    return gather_output(res.results)


# ---------------------------------------------------------------------------
# General-bias fallback (the previous fp32r kernel, correct for any b /
# dense_b). Only used when the fast-path precondition fails.
# ---------------------------------------------------------------------------


def build_nc_bias(t_warm=T, t_dec=OUT_STEPS - 1, bl=BL, reps=None):
    import contextlib

    import concourse.bass as bass  # noqa: F401
    import concourse.mybir as mybir
    import concourse.tile as tile
    from concourse import bacc

    f32 = mybir.dt.float32
    f32r = mybir.dt.float32r
    AF = mybir.ActivationFunctionType
    n_out = t_dec + 1

    nc = bacc.Bacc("TRN2", target_bir_lowering=False, debug=False,
                   num_devices=N_CORES)

    xT_d = nc.dram_tensor("xT", [t_warm, F + 1, bl], f32,
                          kind="ExternalInput").ap()
    wx_d = nc.dram_tensor("wx_aug", [F + 1, 4 * U], f32,
                          kind="ExternalInput").ap()
    wh_d = nc.dram_tensor("wh", [U, 4 * U], f32, kind="ExternalInput").ap()
    whd_d = nc.dram_tensor("wh_dec", [U, 4 * U], f32,
                           kind="ExternalInput").ap()
    bdec_d = nc.dram_tensor("b_dec", [1, 4 * U], f32,
                            kind="ExternalInput").ap()
    dw_d = nc.dram_tensor("dense_W", [U, F], f32, kind="ExternalInput").ap()
    db_d = nc.dram_tensor("dense_b", [F, 1], f32, kind="ExternalInput").ap()
    out_d = nc.dram_tensor("outT", [n_out, F, bl], f32,
                           kind="ExternalOutput").ap()
    H_d = nc.dram_tensor("H", [n_out, 128, 4 * bl], f32r).ap()

    with tile.TileContext(nc) as tc:
        with (
            tc.tile_pool(name="wpool", bufs=1) as wpool,
            tc.tile_pool(name="state", bufs=1) as state,
        ):
            with tc.tile_pool(name="staging", bufs=1) as staging:
                wh_f = staging.tile([128, 4, 4 * U], f32, tag="big")
                nc.sync.dma_start(out=wh_f,
                                  in_=wh_d.rearrange("(k p) n -> p k n", p=128))
                wh_r = wpool.tile([128, 4, 4 * U], f32r)
                nc.vector.tensor_copy(wh_r, wh_f)

                whd_f = staging.tile([128, 4, 4 * U], f32, tag="big2")
                nc.sync.dma_start(out=whd_f,
                                  in_=whd_d.rearrange("(k p) n -> p k n", p=128))
                whd_r = wpool.tile([128, 4, 4 * U], f32r)
                nc.vector.tensor_copy(whd_r, whd_f)

                wx_f = staging.tile([F + 1, 4 * U], f32, tag="small")
                nc.sync.dma_start(out=wx_f, in_=wx_d[:, :])
                wx_r = wpool.tile([F + 1, 4 * U], f32r)
                nc.vector.tensor_copy(wx_r, wx_f)

                wxd_f = staging.tile([F + 1, 4 * U], f32, tag="small2")
                nc.vector.memset(wxd_f, 0.0)
                nc.sync.dma_start(out=wxd_f[F:F + 1, :], in_=bdec_d[:, :])
                wxd_r = wpool.tile([F + 1, 4 * U], f32r)
                nc.vector.tensor_copy(wxd_r, wxd_f)

                dw_f = staging.tile([128, 4, F], f32, tag="small3")
                nc.sync.dma_start(out=dw_f,
                                  in_=dw_d.rearrange("(k p) n -> p k n", p=128))
                dw_r = wpool.tile([128, 4, F], f32r)
                nc.vector.tensor_copy(dw_r, dw_f)

                db_sb = wpool.tile([F, 1], f32)
                nc.sync.dma_start(out=db_sb, in_=db_d[:, :])

            xdec_f = wpool.tile([F + 1, bl], f32)
            nc.vector.memset(xdec_f, 0.0)
            nc.vector.memset(xdec_f[F:F + 1, :], 1.0)
            x_dec = wpool.tile([F + 1, bl], f32r)
            nc.vector.tensor_copy(x_dec, xdec_f)

            c_sb = state.tile([128, 4 * bl], f32)
            h_a = state.tile([128, 4 * bl], f32r)
            h_b = state.tile([128, 4 * bl], f32r)
            h_bufs = [h_a, h_b]

            with (
                tc.tile_pool(name="zps", bufs=1, space="PSUM") as zps,
                tc.tile_pool(name="gates", bufs=3) as gates,
                tc.tile_pool(name="tmp", bufs=6) as tmp,
                tc.tile_pool(name="xf", bufs=8) as xf_pool,
                tc.tile_pool(name="xr", bufs=4) as xr_pool,
                tc.tile_pool(name="hload", bufs=6) as hload,
                tc.tile_pool(name="po", bufs=4) as po,
                tc.For_i(0, reps) if reps else contextlib.nullcontext(),
            ):
                nc.vector.memset(c_sb, 0.0)
                nc.vector.tensor_copy(h_a, c_sb)
                xr_tiles = {}

                def fetch_x(t):
                    if t >= t_warm:
                        return
                    x_f = xf_pool.tile([F + 1, bl], f32, tag="xf",
                                       name=f"xf{t}")
                    nc.sync.dma_start(out=x_f, in_=xT_d[t])
                    x_r = xr_pool.tile([F + 1, bl], f32r, tag="xr",
                                       name=f"xr{t}")
                    nc.vector.tensor_copy(x_r, x_f)
                    xr_tiles[t] = x_r

                def step(t, warm):
                    wh = wh_r if warm else whd_r
                    x_r = xr_tiles.pop(t) if warm else x_dec
                    h_rd = h_bufs[t % 2]
                    h_wr = h_bufs[(t + 1) % 2]
                    z = [[zps.tile([128, 2 * bl], f32, tag=f"z{half}{g}",
                                   name=f"z{half}{g}_{t}")
                          for g in range(4)] for half in range(2)]

                    def zt(half, g, q):
                        return z[half][g][:, q * bl:(q + 1) * bl]

                    def wsl(half, g, q):
                        m = 4 * g + 2 * half + q
                        return slice(m * 128, (m + 1) * 128)

                    def xmm(half, g, q, start):
                        wx = wx_r if warm else wxd_r
                        nc.tensor.matmul(
                            zt(half, g, q), wx[:, wsl(half, g, q)],
                            x_r, start=start, stop=False)

                    def hmm(half, g, q, k, stop=False, start=False):
                        nc.tensor.matmul(
                            zt(half, g, q), wh[:, k, wsl(half, g, q)],
                            h_rd[:, k * bl:(k + 1) * bl],
                            start=start, stop=stop)

                    for g in range(4):
                        for q in range(2):
                            xmm(0, g, q, start=(q == 0))
                    for half in range(2):
                        for g in range(4):
                            for q in range(2):
                                hmm(half, g, q, 0,
                                    start=(half == 1 and q == 0))
                    for g in range(4):
                        for q in range(2):
                            xmm(1, g, q, start=False)
                    for half in range(2):
                        for g in range(4):
                            for q in range(2):
                                for k in (1, 2, 3):
                                    hmm(half, g, q, k,
                                        stop=(k == 3 and q == 1))
                    i_sb = gates.tile([128, 4 * bl], f32, tag="ig",
                                      name=f"ig{t}")
                    f_sb = gates.tile([128, 4 * bl], f32, tag="fg",
                                      name=f"fg{t}")
                    g_sb = gates.tile([128, 4 * bl], f32, tag="gg",
                                      name=f"gg{t}")
                    o_sb = gates.tile([128, 4 * bl], f32, tag="og",
                                      name=f"og{t}")
                    for half in range(2):
                        s = slice(half * 2 * bl, (half + 1) * 2 * bl)
                        nc.scalar.activation(i_sb[:, s], z[half][0],
                                             AF.Sigmoid)
                        nc.scalar.activation(f_sb[:, s], z[half][1],
                                             AF.Sigmoid)
                        nc.scalar.activation(g_sb[:, s], z[half][2],
                                             AF.Tanh)
                        nc.scalar.activation(o_sb[:, s], z[half][3],
                                             AF.Sigmoid)
                        t1 = tmp.tile([128, 2 * bl], f32, tag="t1",
                                      name=f"t1_{t}_{half}")
                        nc.vector.tensor_mul(t1, i_sb[:, s], g_sb[:, s])
                        nc.vector.tensor_mul(c_sb[:, s], f_sb[:, s],
                                             c_sb[:, s])
                        nc.vector.tensor_add(c_sb[:, s], c_sb[:, s], t1)
                        tch = tmp.tile([128, 2 * bl], f32, tag="tc",
                                       name=f"tc_{t}_{half}")
                        nc.scalar.activation(tch, c_sb[:, s], AF.Tanh)
                        nc.vector.tensor_mul(h_wr[:, s], o_sb[:, s], tch)
                    return h_wr

                h_cur = h_a
                fetch_x(0)
                fetch_x(1)
                for t in range(t_warm):
                    h_cur = step(t, warm=True)
                    fetch_x(t + 2)
                nc.sync.dma_start(out=H_d[0], in_=h_cur)
                for t in range(1, t_dec + 1):
                    h_cur = step(t_warm + t - 1, warm=False)
                    nc.sync.dma_start(out=H_d[t], in_=h_cur)

                for t in range(n_out):
                    hl = hload.tile([128, 4 * bl], f32r, tag="hl",
                                    name=f"hl{t}")
                    eng = nc.sync if t % 2 == 0 else nc.gpsimd
                    eng.dma_start(out=hl, in_=H_d[t])
                    pps = zps.tile([F, bl], f32, tag=("z00" if t % 2 == 0
                                                      else "z01"),
                                   name=f"pps{t}")
                    for k in range(4):
                        nc.tensor.matmul(pps, dw_r[:, k, :],
                                         hl[:, k * bl:(k + 1) * bl],
                                         start=(k == 0), stop=(k == 3))
                    p_sb = po.tile([F, bl], f32, tag="po", name=f"po{t}")
                    nc.scalar.activation(p_sb, pps, AF.Identity,
                                         bias=db_sb[:, 0:1])
                    nc.sync.dma_start(out=out_d[t], in_=p_sb)

    nc.compile()
    return nc


def prep_inputs_bias(inputs, W_x, W_h, b, dense_W, dense_b, t_warm=T, bl=BL):
    n_cores = inputs.shape[0] // bl
    W_x = np.asarray(W_x, np.float32)
    W_h = np.asarray(W_h, np.float32)
    b = np.asarray(b, np.float32)
    dense_W = np.asarray(dense_W, np.float32)
    dense_b = np.asarray(dense_b, np.float32)

    wx_aug = np.concatenate([W_x, b[None, :]], axis=0)
    wh_dec = (W_h.astype(np.float64)
              + dense_W.astype(np.float64) @ W_x.astype(np.float64)
              ).astype(np.float32)
    b_dec = (b.astype(np.float64)
             + dense_b.astype(np.float64) @ W_x.astype(np.float64)
             ).astype(np.float32)[None, :]

    shared = {
        "wx_aug": wx_aug,
        "wh": W_h,
        "wh_dec": wh_dec,
        "b_dec": b_dec,
        "dense_W": dense_W,
        "dense_b": dense_b[:, None].astype(np.float32),
    }
    in_maps = []
    x = np.asarray(inputs, np.float32)
    for c in range(n_cores):
        shard = x[c * bl:(c + 1) * bl, :t_warm]
        xT = np.ascontiguousarray(shard.transpose(1, 2, 0))
        ones = np.ones((t_warm, 1, bl), np.float32)
        xT_aug = np.ascontiguousarray(np.concatenate([xT, ones], axis=1))
        in_maps.append({"xT": xT_aug, **shared})
    return in_maps


def _kernel_bias(inputs, W_x, W_h, b, dense_W, dense_b):
    from concourse.bass_utils import run_bass_kernel_spmd

    if "nc_bias" not in _CACHE:
        _CACHE["nc_bias"] = build_nc_bias()
    nc = _CACHE["nc_bias"]
    in_maps = prep_inputs_bias(inputs, W_x, W_h, b, dense_W, dense_b)
    res = run_bass_kernel_spmd(nc, in_maps, core_ids=list(range(N_CORES)),
                               trace=False)
    return gather_output(res.results)
